# revision 7
# baseline (speedup 1.0000x reference)
"""AdditiveRelationalGraphConvolution on 8 TRN2 NeuronCores.

out = relu(mean_s(features[neighbors]) @ W.T + mean_s(RWT[relations]))

Data-parallel over batch (4096 rows/core); feature table replicated (bf16).

The kernel is Q7-descriptor-generation bound (~3.2ns per gathered row), so
the design minimizes gather descriptor count:
  - neighbor rows are fetched with dma_gather (int16 indices) from 4 static
    windows of <=32768 rows; each bucket list is quota-padded with a valid
    dummy index (dead slots masked via owner tag 255). Gathered slots land
    at dst[i%128, i//128]; per-slot owner tags (batch row, or 255) let the
    device rebuild one-hot selection matrices (DVE is_equal) and aggregate
    with PE matmuls: aggT[i,b] += G[p,i]*sel[p,b].
  - relations need NO gather: the 238-row relation table lives in SBUF and
    the host ships a per-tile count matrix cnt[r,b] = #occurrences/16; two
    PE matmuls accumulate cnt.T @ RWT straight into the output PSUM.
  - main transform: psum[b,o] = aggT.T @ (W.T/16) + cnt-term, relu on ACT,
    store bf16 (host upcasts to f32).
"""

import sys

sys.path.insert(0, "/opt/trn_rl_repo")

import numpy as np

N_CORES = 8
B = 32768
S = 16
D = 256
NUM_NODES = 100000
NUM_REL = 238
B_LOC = B // N_CORES  # 4096
P = 128
TILES = B_LOC // P  # 32

# feature-index windows (int16 range)
WIN = [(0, 32768), (32768, 65536), (65536, 98304), (98304, 100000)]
DEAD = 255.0

_CACHE = {}


def _build(QUOTA):
    import concourse.bass as bass
    import concourse.tile as tile
    from concourse import bacc, mybir

    NCHUNK = [q // P for q in QUOTA]
    CHUNKS = sum(NCHUNK)
    f32 = mybir.dt.float32
    bf16 = mybir.dt.bfloat16
    i16 = mybir.dt.int16

    nc = bacc.Bacc(
        "TRN2",
        target_bir_lowering=False,
        debug=False,
        enable_asserts=False,
        num_devices=N_CORES,
        num_swdge_queues=4,
        dynamic_dma_scratch_size=49152,
    )
    feat = nc.dram_tensor("feat", [NUM_NODES, D], bf16, kind="ExternalInput").ap()
    rwt = nc.dram_tensor("rwt", [2 * P, D], bf16, kind="ExternalInput").ap()
    wT = nc.dram_tensor("wT", [D, D], bf16, kind="ExternalInput").ap()
    # per-tile concatenated per-bucket wrapped int16 index lists (4 windows)
    IDXCOLS = sum(QUOTA) // 16
    nidx = nc.dram_tensor("nidx", [P, TILES * IDXCOLS], i16, kind="ExternalInput").ap()
    owner = nc.dram_tensor(
        "owner", [P, TILES * CHUNKS], bf16, kind="ExternalInput"
    ).ap()
    iota = nc.dram_tensor("iota", [P, P], bf16, kind="ExternalInput").ap()
    cnt = nc.dram_tensor("cnt", [P, TILES * 2 * P], bf16, kind="ExternalInput").ap()
    out = nc.dram_tensor("out", [B_LOC, D], bf16, kind="ExternalOutput").ap()

    with tile.TileContext(nc) as tc:
        with (
            tc.tile_pool(name="const", bufs=1) as cp,
            tc.tile_pool(name="gfix", bufs=2) as gfix,
            tc.tile_pool(name="sel", bufs=2) as selp,
            tc.tile_pool(name="small", bufs=3) as small,
            tc.tile_pool(name="psA", bufs=2, space="PSUM") as psA,
            tc.tile_pool(name="psB", bufs=2, space="PSUM") as psB,
        ):
            wt_sb = cp.tile([P, 2 * D], bf16)
            nc.sync.dma_start(out=wt_sb[:, 0:D], in_=wT[0:P, :])
            nc.sync.dma_start(out=wt_sb[:, D : 2 * D], in_=wT[P : 2 * P, :])
            rwt_sb = cp.tile([P, 2 * D], bf16)
            nc.sync.dma_start(out=rwt_sb[:, 0:D], in_=rwt[0:P, :])
            nc.sync.dma_start(out=rwt_sb[:, D : 2 * D], in_=rwt[P : 2 * P, :])
            iota3_sb = cp.tile([P, P], bf16)
            nc.sync.dma_start(out=iota3_sb[:], in_=iota[:])

            qctr = [0]

            def nextq():
                q = qctr[0] % 4
                qctr[0] += 1
                return q

            GRP = 4  # tiles per gather group (quota-aligned concatenation)
            Gg = {}
            OWG = {}
            CNTG = {}
            for tg in range(TILES // GRP):
                # stream this group's index/owner/cnt slices (small HWDGE
                # loads, prefetched a group ahead) instead of one big
                # upfront load that stalls the first gathers.
                nidx_sb = gfix.tile(
                    [P, GRP * IDXCOLS], i16, name=f"nidx{tg}", tag="nidx", bufs=3
                )
                nc.scalar.dma_start(
                    out=nidx_sb[:],
                    in_=nidx[:, tg * GRP * IDXCOLS : (tg + 1) * GRP * IDXCOLS],
                )
                ow_sb = gfix.tile(
                    [P, GRP * CHUNKS], bf16, name=f"ow{tg}", tag="ow", bufs=3
                )
                nc.scalar.dma_start(
                    out=ow_sb[:],
                    in_=owner[:, tg * GRP * CHUNKS : (tg + 1) * GRP * CHUNKS],
                )
                cnt_sb = gfix.tile(
                    [P, GRP * 2 * P], bf16, name=f"cnt{tg}", tag="cnt", bufs=3
                )
                nc.scalar.dma_start(
                    out=cnt_sb[:],
                    in_=cnt[:, tg * GRP * 2 * P : (tg + 1) * GRP * 2 * P],
                )
                OWG[tg] = ow_sb
                CNTG[tg] = cnt_sb

                off = 0
                G = []
                for k in range(4):
                    w = GRP * QUOTA[k] // 16
                    g = gfix.tile(
                        [P, GRP * NCHUNK[k] * D],
                        bf16,
                        name=f"g{tg}_{k}",
                        tag=f"gath{k}",
                        bufs=2,
                    )
                    nc.gpsimd.dma_gather(
                        out_ap=g[:].rearrange("p (c d) -> p c d", d=D),
                        in_ap=feat[WIN[k][0] : WIN[k][1], :],
                        idxs_ap=nidx_sb[:, off : off + w],
                        num_idxs=GRP * QUOTA[k],
                        num_idxs_reg=GRP * QUOTA[k],
                        elem_size=D,
                        single_packet=False,
                        queue_num=nextq(),
                    )
                    off += w
                    G.append(g)
                Gg[tg] = G

                for t in range(tg * GRP, (tg + 1) * GRP):
                    ti = t % GRP  # tile index within group

                    # all one-hot selection matrices in one broadcast op:
                    # sel[p, c, b] = (owner[p, c] == iota[b])
                    sel = selp.tile([P, CHUNKS * P], bf16, tag="sel")
                    ow = OWG[tg][:, ti * CHUNKS : (ti + 1) * CHUNKS]
                    nc.vector.tensor_tensor(
                        out=sel[:].rearrange("p (c b) -> p c b", b=P),
                        in0=ow[:, :, None].to_broadcast([P, CHUNKS, P]),
                        in1=iota3_sb[:, None, :].to_broadcast([P, CHUNKS, P]),
                        op=mybir.AluOpType.is_equal,
                    )

                    # aggT[i, b] = sum_p G[p, i] * sel[p, b] over all nbr chunks
                    agT0 = psA.tile([P, P], f32, tag="agT0", space="PSUM")
                    agT1 = psA.tile([P, P], f32, tag="agT1", space="PSUM")
                    ci = 0
                    for k in range(4):
                        for lc in range(NCHUNK[k]):
                            lhs_t = Gg[tg][k]
                            gc = ti * NCHUNK[k] + lc
                            for ic, agT in enumerate((agT0, agT1)):
                                nc.tensor.matmul(
                                    out=agT[:],
                                    lhsT=lhs_t[
                                        :, gc * D + ic * P : gc * D + (ic + 1) * P
                                    ],
                                    rhs=sel[:, ci * P : (ci + 1) * P],
                                    start=(ci == 0),
                                    stop=(ci == CHUNKS - 1),
                                )
                            ci += 1
                    aggT = small.tile([P, 2 * P], bf16, tag="aggT")
                    nc.vector.tensor_copy(out=aggT[:, 0:P], in_=agT0[:])
                    nc.vector.tensor_copy(out=aggT[:, P : 2 * P], in_=agT1[:])

                    pm = psB.tile([P, D], f32, tag="pm", space="PSUM")
                    nc.tensor.matmul(
                        out=pm[:],
                        lhsT=aggT[:, 0:P],
                        rhs=wt_sb[:, 0:D],
                        start=True,
                        stop=False,
                    )
                    nc.tensor.matmul(
                        out=pm[:],
                        lhsT=aggT[:, P : 2 * P],
                        rhs=wt_sb[:, D : 2 * D],
                        start=False,
                        stop=False,
                    )
                    # relation term: pm[b,o] += sum_r cnt[r, b] * rwt[r, o]
                    for c in range(2):
                        nc.tensor.matmul(
                            out=pm[:],
                            lhsT=CNTG[tg][:, (ti * 2 + c) * P : (ti * 2 + c + 1) * P],
                            rhs=rwt_sb[:, c * D : (c + 1) * D],
                            start=False,
                            stop=(c == 1),
                        )
                    osb = small.tile([P, D], bf16, tag="osb")
                    nc.scalar.activation(
                        out=osb[:], in_=pm[:], func=mybir.ActivationFunctionType.Relu
                    )
                    nc.sync.dma_start(out=out[t * P : (t + 1) * P, :], in_=osb[:])
    nc.compile()
    return nc


def _get_nc(QUOTA):
    key = ("nc", tuple(QUOTA))
    if key not in _CACHE:
        _CACHE[key] = _build(tuple(QUOTA))
    return _CACHE[key]


def _wrap16(lst, width):
    """Wrap a flat ALL-VALID index list of length width*16 into [128, width]
    int16 (16-partition wrap, replicated to all 8 gpsimd core groups).
    Negative indices + multiple in-flight gathers crash the Q7 ucode, so
    callers must pad with a valid dummy index instead."""
    n = len(lst)
    assert n == width * 16
    outw = np.asarray(lst, dtype=np.int16).reshape(width, 16).T
    return np.tile(outw, (8, 1))


def _quotas_for(neighbors):
    """Smallest 128-multiple quota per bucket covering the actual input."""
    nb = np.ascontiguousarray(neighbors, dtype=np.int64).reshape(
        N_CORES * TILES, P * S
    )
    quotas = []
    for k in range(4):
        cnts = ((nb >= WIN[k][0]) & (nb < WIN[k][1])).sum(axis=1)
        q = int(-(-max(1, cnts.max()) // P) * P)
        quotas.append(q)
    return tuple(quotas)


def _prep_inputs(neighbors, relations, features, weight, relation_weight, QUOTA):
    import ml_dtypes

    NCHUNK = [q // P for q in QUOTA]
    CHUNKS = sum(NCHUNK)

    bf16 = ml_dtypes.bfloat16
    inv_s = np.float32(1.0 / S)

    nb = np.ascontiguousarray(neighbors, dtype=np.int64).reshape(N_CORES, TILES, P, S)
    rl = np.ascontiguousarray(relations, dtype=np.int64).reshape(N_CORES, TILES, P, S)
    feat = np.ascontiguousarray(features.astype(bf16))
    # relation table resident in SBUF: row r = relation_weight[:, r], pad to 256
    rwt_full = np.zeros((2 * P, D), dtype=np.float32)
    rwt_full[:NUM_REL] = relation_weight.T.astype(np.float32)
    rwt = np.ascontiguousarray(rwt_full.astype(bf16))
    wT = np.ascontiguousarray((weight.T.astype(np.float32) * inv_s).astype(bf16))
    iota = np.ascontiguousarray(
        np.broadcast_to(np.arange(P, dtype=np.float32), (P, P)).astype(bf16)
    )

    IDXCOLS = sum(QUOTA) // 16
    in_maps = []
    for core in range(N_CORES):
        nidx = np.zeros((P, TILES * IDXCOLS), dtype=np.int16)
        owner = np.full((P, TILES * CHUNKS), DEAD, dtype=np.float32)
        cnt = np.zeros((P, TILES * 2 * P), dtype=np.float32)
        GRP = 4
        IDXG = GRP * IDXCOLS  # cols per group
        for t in range(TILES):
            tg, ti = t // GRP, t % GRP
            idxs = nb[core, t].ravel()  # j = b*16+s
            owners_flat = np.repeat(np.arange(P), S)
            goff = 0
            cbase = 0
            for k in range(4):
                m = (idxs >= WIN[k][0]) & (idxs < WIN[k][1])
                li = idxs[m] - WIN[k][0]
                lo = owners_flat[m]
                order = np.argsort(li, kind="stable")  # ascending HBM addresses
                li = li[order]
                lo = lo[order]
                cnt_k = len(li)
                assert cnt_k <= QUOTA[k], f"bucket {k} overflow: {cnt_k} > {QUOTA[k]}"
                w = QUOTA[k] // 16
                lpad = np.zeros(QUOTA[k], dtype=np.int16)
                lpad[:cnt_k] = li
                c0 = tg * IDXG + goff * GRP + ti * w
                nidx[:, c0 : c0 + w] = _wrap16(lpad, w)
                goff += w
                # owner per slot: slot i -> (p=i%128, chunk=i//128)
                ow = np.full(QUOTA[k], DEAD, dtype=np.float32)
                ow[:cnt_k] = lo
                owner[
                    :, t * CHUNKS + cbase : t * CHUNKS + cbase + NCHUNK[k]
                ] = ow.reshape(NCHUNK[k], P).T
                cbase += NCHUNK[k]
            # relation count matrix: cnt[r%128, t*256 + (r//128)*128 + b]
            rt = rl[core, t]  # [128 b, 16 s]
            counts = (
                np.bincount(
                    rt.ravel() * P + np.repeat(np.arange(P), S),
                    minlength=2 * P * P,
                )
                .reshape(2 * P, P)
                .astype(np.float32)
            )
            cnt[:, t * 2 * P : t * 2 * P + P] = counts[:P] * inv_s
            cnt[:, t * 2 * P + P : (t + 1) * 2 * P] = counts[P:] * inv_s
        in_maps.append(
            {
                "feat": feat,
                "rwt": rwt,
                "wT": wT,
                "nidx": nidx,
                "owner": owner.astype(bf16),
                "iota": iota,
                "cnt": cnt.astype(bf16),
            }
        )
    return in_maps


def run(in_maps, QUOTA, trace=False, tmpdir=None):
    from concourse.bass_utils import run_bass_kernel_spmd

    nc = _get_nc(QUOTA)
    res = run_bass_kernel_spmd(
        nc, in_maps, core_ids=list(range(N_CORES)), trace=trace, tmpdir=tmpdir
    )
    out = np.concatenate([res.results[i]["out"] for i in range(N_CORES)], axis=0)
    return out.astype(np.float32), res


def kernel(neighbors, relations, features, weight, relation_weight):
    QUOTA = _quotas_for(neighbors)
    in_maps = _prep_inputs(
        neighbors, relations, features, weight, relation_weight, QUOTA
    )
    out, _ = run(in_maps, QUOTA, trace=False)
    return out


# revision 8
# speedup vs baseline: 1.7655x; 1.7655x over previous
"""AdditiveRelationalGraphConvolution on 8 TRN2 NeuronCores.

out = relu(mean_s(features[neighbors]) @ W.T + mean_s(RWT[relations]))

Data-parallel over batch (4096 rows/core); feature table replicated (bf16).

The kernel is Q7-descriptor-generation bound (~3.2ns per gathered row), so
the design minimizes gather descriptor count:
  - neighbor rows are fetched with dma_gather (int16 indices) from 4 static
    windows of <=32768 rows; each bucket list is quota-padded with a valid
    dummy index (dead slots masked via owner tag 255). Gathered slots land
    at dst[i%128, i//128]; per-slot owner tags (batch row, or 255) let the
    device rebuild one-hot selection matrices (DVE is_equal) and aggregate
    with PE matmuls: aggT[i,b] += G[p,i]*sel[p,b].
  - relations need NO gather: the 238-row relation table lives in SBUF and
    the host ships a per-tile count matrix cnt[r,b] = #occurrences/16; two
    PE matmuls accumulate cnt.T @ RWT straight into the output PSUM.
  - main transform: psum[b,o] = aggT.T @ (W.T/16) + cnt-term, relu on ACT,
    store bf16 (host upcasts to f32).
"""

import sys

sys.path.insert(0, "/opt/trn_rl_repo")

import numpy as np

N_CORES = 8
B = 32768
S = 16
D = 256
NUM_NODES = 100000
NUM_REL = 238
B_LOC = B // N_CORES  # 4096
P = 128
TILES = B_LOC // P  # 32

# feature-index windows (int16 range)
WIN = [(0, 32768), (32768, 65536), (65536, 98304), (98304, 100000)]
DEAD = 255.0

_CACHE = {}


def _build(QUOTA):
    import concourse.bass as bass
    import concourse.tile as tile
    from concourse import bacc, mybir

    NCHUNK = [q // P for q in QUOTA]
    CHUNKS = sum(NCHUNK)
    f32 = mybir.dt.float32
    bf16 = mybir.dt.bfloat16
    i16 = mybir.dt.int16

    nc = bacc.Bacc(
        "TRN2",
        target_bir_lowering=False,
        debug=False,
        enable_asserts=False,
        num_devices=N_CORES,
        num_swdge_queues=4,
        dynamic_dma_scratch_size=49152,
    )
    feat = nc.dram_tensor("feat", [NUM_NODES, D], bf16, kind="ExternalInput").ap()
    rwt = nc.dram_tensor("rwt", [2 * P, D], bf16, kind="ExternalInput").ap()
    wT = nc.dram_tensor("wT", [D, D], bf16, kind="ExternalInput").ap()
    # per-tile concatenated per-bucket wrapped int16 index lists (4 windows)
    IDXCOLS = sum(QUOTA) // 16
    nidx = nc.dram_tensor("nidx", [P, TILES * IDXCOLS], i16, kind="ExternalInput").ap()
    owner = nc.dram_tensor(
        "owner", [P, TILES * CHUNKS], bf16, kind="ExternalInput"
    ).ap()
    iota = nc.dram_tensor("iota", [P, P], bf16, kind="ExternalInput").ap()
    cnt = nc.dram_tensor("cnt", [P, TILES * 2 * P], bf16, kind="ExternalInput").ap()
    out = nc.dram_tensor("out", [B_LOC, D], bf16, kind="ExternalOutput").ap()

    with tile.TileContext(nc) as tc:
        with (
            tc.tile_pool(name="const", bufs=1) as cp,
            tc.tile_pool(name="gfix", bufs=2) as gfix,
            tc.tile_pool(name="sel", bufs=2) as selp,
            tc.tile_pool(name="small", bufs=3) as small,
            tc.tile_pool(name="psA", bufs=2, space="PSUM") as psA,
            tc.tile_pool(name="psB", bufs=2, space="PSUM") as psB,
        ):
            nidx_sb = cp.tile([P, TILES * IDXCOLS], i16)
            nc.sync.dma_start(out=nidx_sb[:], in_=nidx[:])
            owner_sb = cp.tile([P, TILES * CHUNKS], bf16)
            nc.sync.dma_start(out=owner_sb[:], in_=owner[:])
            iota3_sb = cp.tile([P, P], bf16)
            nc.sync.dma_start(out=iota3_sb[:], in_=iota[:])
            wt_sb = cp.tile([P, 2 * D], bf16)
            nc.sync.dma_start(out=wt_sb[:, 0:D], in_=wT[0:P, :])
            nc.sync.dma_start(out=wt_sb[:, D : 2 * D], in_=wT[P : 2 * P, :])
            rwt_sb = cp.tile([P, 2 * D], bf16)
            nc.sync.dma_start(out=rwt_sb[:, 0:D], in_=rwt[0:P, :])
            nc.sync.dma_start(out=rwt_sb[:, D : 2 * D], in_=rwt[P : 2 * P, :])
            cnt_sb = cp.tile([P, TILES * 2 * P], bf16)
            nc.sync.dma_start(out=cnt_sb[:], in_=cnt[:])

            qctr = [0]

            def nextq():
                q = qctr[0] % 4
                qctr[0] += 1
                return q

            GRP = 4  # tiles per gather group (quota-aligned concatenation)
            Gg = {}
            for tg in range(TILES // GRP):
                off = 0
                G = []
                for k in range(4):
                    w = GRP * QUOTA[k] // 16
                    g = gfix.tile(
                        [P, GRP * NCHUNK[k] * D],
                        bf16,
                        name=f"g{tg}_{k}",
                        tag=f"gath{k}",
                        bufs=3,
                    )
                    nc.gpsimd.dma_gather(
                        out_ap=g[:].rearrange("p (c d) -> p c d", d=D),
                        in_ap=feat[WIN[k][0] : WIN[k][1], :],
                        idxs_ap=nidx_sb[:, tg * GRP * IDXCOLS + off : tg * GRP * IDXCOLS + off + w],
                        num_idxs=GRP * QUOTA[k],
                        num_idxs_reg=GRP * QUOTA[k],
                        elem_size=D,
                        single_packet=False,
                        queue_num=nextq(),
                    )
                    off += w
                    G.append(g)
                Gg[tg] = G

                for t in range(tg * GRP, (tg + 1) * GRP):
                    ti = t % GRP  # tile index within group

                    # all one-hot selection matrices in one broadcast op:
                    # sel[p, c, b] = (owner[p, c] == iota[b])
                    sel = selp.tile([P, CHUNKS * P], bf16, tag="sel")
                    ow = owner_sb[:, t * CHUNKS : (t + 1) * CHUNKS]
                    nc.vector.tensor_tensor(
                        out=sel[:].rearrange("p (c b) -> p c b", b=P),
                        in0=ow[:, :, None].to_broadcast([P, CHUNKS, P]),
                        in1=iota3_sb[:, None, :].to_broadcast([P, CHUNKS, P]),
                        op=mybir.AluOpType.is_equal,
                    )

                    # aggT[i, b] = sum_p G[p, i] * sel[p, b] over all nbr chunks
                    agT0 = psA.tile([P, P], f32, tag="agT0", space="PSUM")
                    agT1 = psA.tile([P, P], f32, tag="agT1", space="PSUM")
                    ci = 0
                    for k in range(4):
                        for lc in range(NCHUNK[k]):
                            lhs_t = Gg[tg][k]
                            gc = ti * NCHUNK[k] + lc
                            for ic, agT in enumerate((agT0, agT1)):
                                nc.tensor.matmul(
                                    out=agT[:],
                                    lhsT=lhs_t[
                                        :, gc * D + ic * P : gc * D + (ic + 1) * P
                                    ],
                                    rhs=sel[:, ci * P : (ci + 1) * P],
                                    start=(ci == 0),
                                    stop=(ci == CHUNKS - 1),
                                )
                            ci += 1
                    aggT = small.tile([P, 2 * P], bf16, tag="aggT")
                    nc.vector.tensor_copy(out=aggT[:, 0:P], in_=agT0[:])
                    nc.vector.tensor_copy(out=aggT[:, P : 2 * P], in_=agT1[:])

                    pm = psB.tile([P, D], f32, tag="pm", space="PSUM")
                    nc.tensor.matmul(
                        out=pm[:],
                        lhsT=aggT[:, 0:P],
                        rhs=wt_sb[:, 0:D],
                        start=True,
                        stop=False,
                    )
                    nc.tensor.matmul(
                        out=pm[:],
                        lhsT=aggT[:, P : 2 * P],
                        rhs=wt_sb[:, D : 2 * D],
                        start=False,
                        stop=False,
                    )
                    # relation term: pm[b,o] += sum_r cnt[r, b] * rwt[r, o]
                    for c in range(2):
                        nc.tensor.matmul(
                            out=pm[:],
                            lhsT=cnt_sb[:, (t * 2 + c) * P : (t * 2 + c + 1) * P],
                            rhs=rwt_sb[:, c * D : (c + 1) * D],
                            start=False,
                            stop=(c == 1),
                        )
                    osb = small.tile([P, D], bf16, tag="osb")
                    nc.scalar.activation(
                        out=osb[:], in_=pm[:], func=mybir.ActivationFunctionType.Relu
                    )
                    nc.sync.dma_start(out=out[t * P : (t + 1) * P, :], in_=osb[:])
    nc.compile()
    return nc


def _get_nc(QUOTA):
    key = ("nc", tuple(QUOTA))
    if key not in _CACHE:
        _CACHE[key] = _build(tuple(QUOTA))
    return _CACHE[key]


def _wrap16(lst, width):
    """Wrap a flat ALL-VALID index list of length width*16 into [128, width]
    int16 (16-partition wrap, replicated to all 8 gpsimd core groups).
    Negative indices + multiple in-flight gathers crash the Q7 ucode, so
    callers must pad with a valid dummy index instead."""
    n = len(lst)
    assert n == width * 16
    outw = np.asarray(lst, dtype=np.int16).reshape(width, 16).T
    return np.tile(outw, (8, 1))


def _quotas_for(neighbors):
    """Smallest 128-multiple quota per bucket covering the actual input."""
    nb = np.ascontiguousarray(neighbors, dtype=np.int64).reshape(
        N_CORES * TILES, P * S
    )
    quotas = []
    for k in range(4):
        cnts = ((nb >= WIN[k][0]) & (nb < WIN[k][1])).sum(axis=1)
        q = int(-(-max(1, cnts.max()) // P) * P)
        quotas.append(q)
    return tuple(quotas)


def _prep_inputs(neighbors, relations, features, weight, relation_weight, QUOTA):
    import ml_dtypes

    NCHUNK = [q // P for q in QUOTA]
    CHUNKS = sum(NCHUNK)

    bf16 = ml_dtypes.bfloat16
    inv_s = np.float32(1.0 / S)

    nb = np.ascontiguousarray(neighbors, dtype=np.int64).reshape(N_CORES, TILES, P, S)
    rl = np.ascontiguousarray(relations, dtype=np.int64).reshape(N_CORES, TILES, P, S)
    feat = np.ascontiguousarray(features.astype(bf16))
    # relation table resident in SBUF: row r = relation_weight[:, r], pad to 256
    rwt_full = np.zeros((2 * P, D), dtype=np.float32)
    rwt_full[:NUM_REL] = relation_weight.T.astype(np.float32)
    rwt = np.ascontiguousarray(rwt_full.astype(bf16))
    wT = np.ascontiguousarray((weight.T.astype(np.float32) * inv_s).astype(bf16))
    iota = np.ascontiguousarray(
        np.broadcast_to(np.arange(P, dtype=np.float32), (P, P)).astype(bf16)
    )

    IDXCOLS = sum(QUOTA) // 16
    in_maps = []
    for core in range(N_CORES):
        nidx = np.zeros((P, TILES * IDXCOLS), dtype=np.int16)
        owner = np.full((P, TILES * CHUNKS), DEAD, dtype=np.float32)
        cnt = np.zeros((P, TILES * 2 * P), dtype=np.float32)
        GRP = 4
        IDXG = GRP * IDXCOLS  # cols per group
        for t in range(TILES):
            tg, ti = t // GRP, t % GRP
            idxs = nb[core, t].ravel()  # j = b*16+s
            owners_flat = np.repeat(np.arange(P), S)
            goff = 0
            cbase = 0
            for k in range(4):
                m = (idxs >= WIN[k][0]) & (idxs < WIN[k][1])
                li = idxs[m] - WIN[k][0]
                lo = owners_flat[m]
                order = np.argsort(li, kind="stable")  # ascending HBM addresses
                li = li[order]
                lo = lo[order]
                cnt_k = len(li)
                assert cnt_k <= QUOTA[k], f"bucket {k} overflow: {cnt_k} > {QUOTA[k]}"
                w = QUOTA[k] // 16
                lpad = np.zeros(QUOTA[k], dtype=np.int16)
                lpad[:cnt_k] = li
                c0 = tg * IDXG + goff * GRP + ti * w
                nidx[:, c0 : c0 + w] = _wrap16(lpad, w)
                goff += w
                # owner per slot: slot i -> (p=i%128, chunk=i//128)
                ow = np.full(QUOTA[k], DEAD, dtype=np.float32)
                ow[:cnt_k] = lo
                owner[
                    :, t * CHUNKS + cbase : t * CHUNKS + cbase + NCHUNK[k]
                ] = ow.reshape(NCHUNK[k], P).T
                cbase += NCHUNK[k]
            # relation count matrix: cnt[r%128, t*256 + (r//128)*128 + b]
            rt = rl[core, t]  # [128 b, 16 s]
            counts = (
                np.bincount(
                    rt.ravel() * P + np.repeat(np.arange(P), S),
                    minlength=2 * P * P,
                )
                .reshape(2 * P, P)
                .astype(np.float32)
            )
            cnt[:, t * 2 * P : t * 2 * P + P] = counts[:P] * inv_s
            cnt[:, t * 2 * P + P : (t + 1) * 2 * P] = counts[P:] * inv_s
        in_maps.append(
            {
                "feat": feat,
                "rwt": rwt,
                "wT": wT,
                "nidx": nidx,
                "owner": owner.astype(bf16),
                "iota": iota,
                "cnt": cnt.astype(bf16),
            }
        )
    return in_maps


def run(in_maps, QUOTA, trace=False, tmpdir=None):
    from concourse.bass_utils import run_bass_kernel_spmd

    nc = _get_nc(QUOTA)
    res = run_bass_kernel_spmd(
        nc, in_maps, core_ids=list(range(N_CORES)), trace=trace, tmpdir=tmpdir
    )
    out = np.concatenate([res.results[i]["out"] for i in range(N_CORES)], axis=0)
    return out.astype(np.float32), res


def kernel(neighbors, relations, features, weight, relation_weight):
    QUOTA = _quotas_for(neighbors)
    in_maps = _prep_inputs(
        neighbors, relations, features, weight, relation_weight, QUOTA
    )
    out, _ = run(in_maps, QUOTA, trace=False)
    return out
